# revision 1
# baseline (speedup 1.0000x reference)
"""BRITS GRU-cell recurrence on 8 Trainium2 NeuronCores — v3.

B=8192, T=256, H=128; data-parallel over 8 cores (bs=1024/core), NT=2
column chains of TN=512. bf16 datapath, fp32 PSUM.

Key structure:
- One ACT op per chain/step computes tanh(small + Wc_b) over BOTH rows
  [clogit; pred] straight into the bf16 output stage: row0 IS comps_t,
  row1 is tanh(pred_{t-1}+Wc_b) which the host inverts with atanh. This
  yields the outputs for free (no PSUM->SBUF copies) and feeds the c row.
- omc: mxc row0 = om * stage row0 (bf16 DVE).
- npre is eliminated: rhn = (h_n+b_hhn)*r (DVE) is accumulated into the
  i_n PSUM bank with an identity matmul on the PE; tanh_n reads i_n.
- relu(featpre+b) alternates ACT (b0) / DVE (b1) for balance.
- h-update: d=h-n, zd=z*d, h=n+zd — three bf16 DVE ops.
- Emission is op-position-major so both chains' same-position ops are
  adjacent in each in-order queue; W_hh/W_mb matmuls issued first.
"""

import os
import sys
from contextlib import ExitStack

import numpy as np

for _p in ("/opt/trn_rl_repo", "/opt/pypackages"):
    if _p not in sys.path and os.path.isdir(_p):
        sys.path.append(_p)

import concourse.bass as bass
import concourse.bacc as bacc
import concourse.tile as tile
from concourse import mybir
from concourse.bass_utils import run_bass_kernel_spmd

B, T, H = 8192, 256, 128
NCORES = 8
BS = B // NCORES  # 1024 samples per core
NT = 2            # chains
TN = BS // NT     # 512
G = 4             # steps per I/O block
F32 = mybir.dt.float32
BF16 = mybir.dt.float16
QD_NP = np.float16


def build_program(t_steps=T, bs=BS):
    assert t_steps % G == 0
    nc = bacc.Bacc("TRN2", target_bir_lowering=False, debug=False)
    gbs = G * bs

    xm = nc.dram_tensor("xm", [t_steps, 3, bs], BF16, kind="ExternalInput").ap()
    wihT = nc.dram_tensor("wihT", [H, 3 * H], BF16, kind="ExternalInput").ap()
    whhT = nc.dram_tensor("whhT", [H, 3 * H], BF16, kind="ExternalInput").ap()
    wmb = nc.dram_tensor("wmb", [H, 3 * H], BF16, kind="ExternalInput").ap()
    wxw = nc.dram_tensor("wxw", [H, H], BF16, kind="ExternalInput").ap()
    wsmall = nc.dram_tensor("wsmall", [H, H], BF16, kind="ExternalInput").ap()
    ident = nc.dram_tensor("ident", [H, H], BF16, kind="ExternalInput").ap()
    biases = nc.dram_tensor("biases", [H, 3], F32, kind="ExternalInput").ap()
    mone = nc.dram_tensor("mone", [1, gbs], BF16, kind="ExternalInput").ap()
    opc = nc.dram_tensor("opc", [t_steps, 2, bs], BF16, kind="ExternalOutput").ap()
    opl = nc.dram_tensor("opl", [2, bs], BF16, kind="ExternalOutput").ap()

    AF = mybir.ActivationFunctionType
    OP = mybir.AluOpType

    with tile.TileContext(nc) as tc, ExitStack() as ctx:
        const = ctx.enter_context(tc.tile_pool(name="const", bufs=1))
        work = ctx.enter_context(tc.tile_pool(name="work", bufs=2))

        # --- constants ---
        w_ih = const.tile([H, 3 * H], BF16)
        nc.sync.dma_start(w_ih[:], wihT[:])
        w_hh = const.tile([H, 3 * H], BF16)
        nc.sync.dma_start(w_hh[:], whhT[:])
        w_mb = const.tile([H, 3 * H], BF16)
        nc.sync.dma_start(w_mb[:], wmb[:])
        w_xw = const.tile([H, H], BF16)
        nc.sync.dma_start(w_xw[:], wxw[:])
        w_sm = const.tile([H, H], BF16)
        nc.sync.dma_start(w_sm[:], wsmall[:])
        w_id = const.tile([H, H], BF16)
        nc.sync.dma_start(w_id[:], ident[:])
        w_z = const.tile([H, H], BF16)
        nc.vector.memset(w_z[:], 0.0)
        bia = const.tile([H, 3], F32)
        nc.sync.dma_start(bia[:], biases[:])
        zrow = const.tile([2, bs], F32)
        nc.vector.memset(zrow[:], 0.0)

        # per-block staged tensors, double buffered
        m2_ab, mxc_ab, om_ab, cp_ab = [], [], [], []
        for i_ in range(2):
            mt = const.tile([H, gbs], BF16, tag=f"m2{i_}", name=f"m2f{i_}")
            nc.vector.memset(mt[:], 0.0)
            nc.sync.dma_start(mt[1:2, :], mone[:])
            m2_ab.append(mt)
            mx = const.tile([H, gbs], BF16, tag=f"mxc{i_}", name=f"mxc{i_}")
            nc.vector.memset(mx[:], 0.0)
            mxc_ab.append(mx)
            om_ab.append(const.tile([1, gbs], BF16, tag=f"om{i_}", name=f"om{i_}"))
            cp_ab.append(const.tile([2, gbs], BF16, tag=f"cp{i_}", name=f"cp{i_}"))
        cp_l = const.tile([2, bs], BF16, tag="cpl")

        # preload I/O block 0
        nc.sync.dma_start(m2_ab[0][0:1, :], xm[0:G, 0, :])
        nc.sync.dma_start(mxc_ab[0][1:2, :], xm[0:G, 1, :])
        nc.sync.dma_start(om_ab[0][0:1, :], xm[0:G, 2, :])

        tc.strict_bb_all_engine_barrier()

        b_hhn = bia[:, 0:1]
        b_wx = bia[:, 1:2]
        b_wc = bia[0:2, 2:3]

        h_b, ps_acc, ps_pfi, ps_phn = [], [], [], []
        for b_ in range(NT):
            hb = const.tile([H, TN], BF16, tag=f"h{b_}", name=f"h{b_}")
            nc.vector.memset(hb[:], 0.0)
            h_b.append(hb)
            ps_acc.append(ctx.enter_context(
                tc.tile_pool(name=f"psacc{b_}", bufs=1, space="PSUM")))
            ps_pfi.append(ctx.enter_context(
                tc.tile_pool(name=f"pspfi{b_}", bufs=1, space="PSUM")))
            ps_phn.append(ctx.enter_context(
                tc.tile_pool(name=f"psphn{b_}", bufs=1, space="PSUM")))

        prev_small = [None] * NT
        pending_hadd = None
        sr, sz_ = slice(0, TN), slice(TN, 2 * TN)

        # acc(0): wmb contribution issued ahead (keeps PE busy in gaps)
        acc_cur = []
        for b_ in range(NT):
            acc_cur.append(ps_acc[b_].tile([H, 2 * TN], F32, tag=f"acc{b_}",
                                           name=f"acc{b_}_0"))
        for b_ in range(NT):
            nc.tensor.matmul(acc_cur[b_][:, sr], w_mb[:, 0:H],
                             m2_ab[0][0:H, b_ * TN:(b_ + 1) * TN],
                             start=True, stop=False)
        for b_ in range(NT):
            nc.tensor.matmul(acc_cur[b_][:, sz_], w_mb[:, H:2 * H],
                             m2_ab[0][0:H, b_ * TN:(b_ + 1) * TN],
                             start=True, stop=False)

        for t in range(t_steps):
            g = t % G
            blk = (t // G) % 2
            off = g * bs
            m2 = m2_ab[blk]
            mxc = mxc_ab[blk]
            cp = cp_ab[blk]
            sob = [slice(off + b * TN, off + (b + 1) * TN) for b in range(NT)]

            # prefetch next I/O block early in this one
            if g == 1:
                t0n = (t // G + 1) * G
                if t0n < t_steps:
                    nblk = (t0n // G) % 2
                    nc.sync.dma_start(m2_ab[nblk][0:1, :], xm[t0n:t0n + G, 0, :])
                    nc.sync.dma_start(mxc_ab[nblk][1:2, :], xm[t0n:t0n + G, 1, :])
                    nc.sync.dma_start(om_ab[nblk][0:1, :], xm[t0n:t0n + G, 2, :])

            # drain previous completed block to HBM
            if g == 0 and t > 0:
                t0 = t - G
                pblk = (t0 // G) % 2
                nc.sync.dma_start(opc[t0:t0 + G, 0, :], cp_ab[pblk][0:1, :])
                nc.sync.dma_start(opc[t0:t0 + G, 1, :], cp_ab[pblk][1:2, :])

            # --- ACT: tanh over [clogit; pred] rows -> output stage ---
            for b in range(NT):
                if prev_small[b] is None:
                    nc.scalar.activation(cp[0:2, sob[b]], zrow[0:2, 0:TN],
                                         AF.Tanh, bias=b_wc)
                else:
                    nc.scalar.activation(cp[0:2, sob[b]], prev_small[b][0:2, :],
                                         AF.Tanh, bias=b_wc)
            # --- DVE: omc rows, with chain b1's deferred h-update add
            #     interleaved so omc(b0) is not queue-blocked behind it ---
            nc.vector.tensor_mul(mxc[0:1, sob[0]], om_ab[blk][0:1, sob[0]],
                                 cp[0:1, sob[0]])
            if pending_hadd is not None:
                nb, nn, nd = pending_hadd
                nc.vector.tensor_add(h_b[nb][:], nn[:], nd[:])
                pending_hadd = None
            nc.vector.tensor_mul(mxc[0:1, sob[1]], om_ab[blk][0:1, sob[1]],
                                 cp[0:1, sob[1]])

            # --- PE: h-dependent matmuls + featpre per chain ---
            acc = acc_cur
            h_n, featpre = [], []
            for b in range(NT):
                h_n.append(ps_phn[b].tile([H, TN], F32, tag=f"hn{b}",
                                          name=f"hn{b}_{t}"))
                featpre.append(ps_pfi[b].tile([H, TN], F32, tag=f"pfi{b}",
                                              name=f"fp{b}_{t}"))
            for b in range(NT):
                nc.tensor.matmul(h_n[b][:], w_hh[:, 2 * H:], h_b[b][:],
                                 start=True, stop=True)
                nc.tensor.matmul(acc[b][:, sr], w_hh[:, 0:H], h_b[b][:],
                                 start=False, stop=False)
                nc.tensor.matmul(acc[b][:, sz_], w_hh[:, H:2 * H], h_b[b][:],
                                 start=False, stop=False)
                nc.tensor.matmul(featpre[b][:], w_xw[:, :], mxc[0:H, sob[b]],
                                 start=True, stop=True)
            feat = []
            for b in range(NT):
                feat.append(work.tile([H, TN], BF16, tag=f"feat{b}",
                                      name=f"ft{b}_{t}"))
            nc.scalar.activation(feat[0][:], featpre[0][:], AF.Relu, bias=b_wx)
            nc.vector.tensor_scalar(feat[1][:], featpre[1][:], b_wx, 0.0,
                                    OP.add, OP.max)

            # --- feat-dependent matmuls ---
            for b in range(NT):
                nc.tensor.matmul(acc[b][:, sr], w_ih[:, 0:H], feat[b][:],
                                 start=False, stop=True)
            for b in range(NT):
                nc.tensor.matmul(acc[b][:, sz_], w_ih[:, H:2 * H], feat[b][:],
                                 start=False, stop=True)
            i_n = []
            for b in range(NT):
                i_n.append(ps_pfi[b].tile([H, TN], F32, tag=f"pfi{b}",
                                          name=f"in{b}_{t}"))
            for b in range(NT):
                nc.tensor.matmul(i_n[b][:], w_mb[:, 2 * H:], m2[0:H, sob[b]],
                                 start=True, stop=False)
            for b in range(NT):
                nc.tensor.matmul(i_n[b][:], w_ih[:, 2 * H:], feat[b][:],
                                 start=False, stop=False)

            # --- gates ---
            r_sb, z_sb = [], []
            for b in range(NT):
                r_sb.append(work.tile([H, TN], BF16, tag=f"r{b}",
                                      name=f"r{b}_{t}"))
                nc.scalar.activation(r_sb[b][:], acc[b][:, sr], AF.Sigmoid)
            for b in range(NT):
                z_sb.append(work.tile([H, TN], BF16, tag=f"z{b}",
                                      name=f"z{b}_{t}"))

            # pull next step's wmb matmuls into this step's sigma window
            if t + 1 < t_steps:
                tn = t + 1
                blkn = (tn // G) % 2
                offn = (tn % G) * bs
                m2n = m2_ab[blkn]
                acc_nx = []
                for b in range(NT):
                    acc_nx.append(ps_acc[b].tile([H, 2 * TN], F32,
                                                 tag=f"acc{b}",
                                                 name=f"acc{b}_{tn}"))
                for b in range(NT):
                    nc.tensor.matmul(acc_nx[b][:, sr], w_mb[:, 0:H],
                                     m2n[0:H, offn + b * TN:offn + (b + 1) * TN],
                                     start=True, stop=False)
                for b in range(NT):
                    nc.tensor.matmul(acc_nx[b][:, sz_], w_mb[:, H:2 * H],
                                     m2n[0:H, offn + b * TN:offn + (b + 1) * TN],
                                     start=True, stop=False)
                acc_cur = acc_nx

            # rhn = (h_n + b_hhn) * r  (DVE), then accumulated into i_n by an
            # identity matmul on the PE; tanh_n reads the completed i_n bank.
            hn16 = []
            for b in range(NT):
                hn16.append(work.tile([H, TN], BF16, tag=f"hn16{b}", bufs=1,
                                      name=f"hc{b}_{t}"))
                nc.vector.tensor_copy(hn16[b][:], h_n[b][:])
            rhn = []
            for b in range(NT):
                rhn.append(work.tile([H, TN], BF16, tag=f"rhn{b}", bufs=1,
                                     name=f"rh{b}_{t}"))
                nc.vector.scalar_tensor_tensor(rhn[b][:], hn16[b][:], b_hhn,
                                               r_sb[b][:], OP.add, OP.mult)
            for b in range(NT):
                nc.tensor.matmul(i_n[b][:], w_id[:, :], rhn[b][:],
                                 start=False, stop=True)
            n_sb = []
            for b in range(NT):
                n_sb.append(work.tile([H, TN], BF16, tag=f"n{b}",
                                      name=f"n{b}_{t}"))
                nc.scalar.activation(z_sb[b][:], acc[b][:, sz_], AF.Sigmoid)
                nc.scalar.activation(n_sb[b][:], i_n[b][:], AF.Tanh)

            # --- h = n + z*(h-n); small = wsm@n + wsm@zd (accumulated) so
            #     the next step's tanh_c only waits for zd, not h' ---
            d_sb, small_t = [], []
            for b in range(NT):
                d_sb.append(work.tile([H, TN], BF16, tag=f"d{b}", bufs=1,
                                      name=f"d{b}_{t}"))
                small_t.append(ps_pfi[b].tile([H, TN], F32, tag=f"pfi{b}",
                                              name=f"sm{b}_{t}"))
            for b in range(NT):
                nc.vector.tensor_sub(d_sb[b][:], h_b[b][:], n_sb[b][:])
                nc.vector.tensor_mul(d_sb[b][:], d_sb[b][:], z_sb[b][:])
            nc.tensor.matmul(small_t[0][:], w_sm[:, :], n_sb[0][:],
                             start=True, stop=False)
            nc.tensor.matmul(small_t[0][:], w_sm[:, :], d_sb[0][:],
                             start=False, stop=True)
            nc.tensor.matmul(small_t[1][:], w_sm[:, :], n_sb[1][:],
                             start=True, stop=False)
            nc.tensor.matmul(small_t[1][:], w_sm[:, :], d_sb[1][:],
                             start=False, stop=True)
            for b in range(NT):
                prev_small[b] = small_t[b]
            nc.vector.tensor_add(h_b[0][:], n_sb[0][:], d_sb[0][:])
            if t + 1 < t_steps:
                pending_hadd = (1, n_sb[1], d_sb[1])
            else:
                nc.vector.tensor_add(h_b[1][:], n_sb[1][:], d_sb[1][:])
            if t + 1 < t_steps:
                for j_ in range(2):
                    nc.tensor.matmul(acc_cur[j_ % NT][:, sr],
                                     w_z[:, :], m2[0:H, sob[j_ % NT]],
                                     start=False, stop=False)

        # final: tanh of last smalls (pred_{T-1}) + remaining DMA
        for b in range(NT):
            nc.scalar.activation(cp_l[0:2, b * TN:(b + 1) * TN],
                                 prev_small[b][0:2, :], AF.Tanh, bias=b_wc)
        nc.sync.dma_start(opl[:, :], cp_l[:])
        t0 = t_steps - G
        pblk = (t0 // G) % 2
        nc.sync.dma_start(opc[t0:t0 + G, 0, :], cp_ab[pblk][0:1, :])
        nc.sync.dma_start(opc[t0:t0 + G, 1, :], cp_ab[pblk][1:2, :])

    nc.compile()
    return nc


def make_in_maps(x_seq, m_seq, Wc_w, Wc_b, Wx_w, Wx_b, W_ih, W_hh, b_ih, b_hh,
                 out_w, out_b, t_steps=T, bs=BS, ncores=NCORES):
    bf = QD_NP
    f = np.float32
    wihT = np.ascontiguousarray(W_ih[:, :H].T).astype(bf)          # [128, 384]
    whhT = np.ascontiguousarray(W_hh.T).astype(bf)                 # [128, 384]
    wmb = np.zeros((H, 3 * H), dtype=f)
    wmb[0] = W_ih[:, H]
    wmb[1, 0:H] = b_ih[0:H] + b_hh[0:H]
    wmb[1, H:2 * H] = b_ih[H:2 * H] + b_hh[H:2 * H]
    wmb[1, 2 * H:] = b_ih[2 * H:]
    wmb = wmb.astype(bf)
    wxw = np.zeros((H, H), dtype=f)
    wxw[0] = Wx_w[:, 0]
    wxw[1] = Wx_w[:, 0]
    wxw = wxw.astype(bf)
    wsmall = np.zeros((H, H), dtype=f)
    wsmall[:, 0] = Wc_w[0]
    wsmall[:, 1] = out_w[0]
    wsmall = wsmall.astype(bf)
    biases = np.zeros((H, 3), dtype=f)
    biases[:, 0] = b_hh[2 * H:]
    biases[:, 1] = Wx_b
    biases[0, 2] = Wc_b[0]
    biases[1, 2] = Wc_b[0]

    xT = np.ascontiguousarray(x_seq.T, dtype=f)  # [T, B]
    mT = np.ascontiguousarray(m_seq.T, dtype=f)

    in_maps = []
    for i in range(ncores):
        sl = slice(i * bs, (i + 1) * bs)
        xmc = np.empty((t_steps, 3, bs), dtype=bf)
        xmc[:, 0, :] = mT[:t_steps, sl].astype(bf)
        xmc[:, 1, :] = (mT[:t_steps, sl] * xT[:t_steps, sl]).astype(bf)
        xmc[:, 2, :] = (1.0 - mT[:t_steps, sl]).astype(bf)
        in_maps.append({
            "xm": xmc, "wihT": wihT, "whhT": whhT, "wmb": wmb, "wxw": wxw,
            "wsmall": wsmall, "biases": biases,
            "ident": np.eye(H, dtype=f).astype(bf),
            "mone": np.ones((1, G * bs), dtype=bf),
        })
    return in_maps


def postprocess(results, Wc_b, out_b, t_steps=T, bs=BS, ncores=NCORES):
    wc = np.float32(Wc_b[0])
    ob = np.float32(out_b[0])
    preds = np.empty((ncores * bs, t_steps), dtype=np.float32)
    comps = np.empty((ncores * bs, t_steps), dtype=np.float32)
    for i in range(ncores):
        o = np.asarray(results[i]["opc"], dtype=np.float32)  # [T, 2, bs]
        ol = np.asarray(results[i]["opl"], dtype=np.float32)  # [2, bs]
        sl = slice(i * bs, (i + 1) * bs)
        comps[sl, :] = o[:, 0, :].T
        tp = np.concatenate([o[1:, 1, :], ol[1:2, :]], axis=0)  # [T, bs]
        preds[sl, :] = np.arctanh(np.clip(tp, -0.9999999, 0.9999999)).T - wc + ob
    return preds, comps


_CACHE = {}


def kernel(x_seq, m_seq, Wc_w, Wc_b, Wx_w, Wx_b, W_ih, W_hh, b_ih, b_hh,
           out_w, out_b):
    Wc_b = np.asarray(Wc_b)
    out_b = np.asarray(out_b)
    x_seq = np.asarray(x_seq, dtype=np.float32)
    m_seq = np.asarray(m_seq, dtype=np.float32)
    if "nc" not in _CACHE:
        _CACHE["nc"] = build_program()
    nc = _CACHE["nc"]
    in_maps = make_in_maps(x_seq, m_seq, np.asarray(Wc_w), Wc_b,
                           np.asarray(Wx_w), np.asarray(Wx_b), np.asarray(W_ih),
                           np.asarray(W_hh), np.asarray(b_ih), np.asarray(b_hh),
                           np.asarray(out_w), out_b)
    res = run_bass_kernel_spmd(nc, in_maps, list(range(NCORES)))
    return postprocess(res.results, Wc_b, out_b)



# revision 2
# speedup vs baseline: 1.0185x; 1.0185x over previous
"""BRITS GRU-cell recurrence on 8 Trainium2 NeuronCores — v10.

B=8192, T=256, H=128; data-parallel over 8 cores (bs=1024/core), NT=2
column chains of TN=512. bf16 datapath, fp32 PSUM.

Key structure:
- One ACT op per chain/step computes tanh(small + Wc_b) over BOTH rows
  [clogit; pred] straight into the bf16 output stage: row0 IS comps_t,
  row1 is tanh(pred_{t-1}+Wc_b) which the host inverts with atanh. This
  yields the outputs for free (no PSUM->SBUF copies) and feeds the c row.
- omc: mxc row0 = om * stage row0 (bf16 DVE).
- npre is eliminated: rhn = (h_n+b_hhn)*r (DVE) is accumulated into the
  i_n PSUM bank with an identity matmul on the PE; tanh_n reads i_n.
- relu(featpre+b) alternates ACT (b0) / DVE (b1) for balance.
- h-update: d=h-n, zd=z*d, h=n+zd — three bf16 DVE ops.
- Emission is op-position-major so both chains' same-position ops are
  adjacent in each in-order queue; W_hh/W_mb matmuls issued first.
- The four (m,bias) rank-2 pre-issue matmuls are K=2, row-group packed at
  tile positions 0/64 (r) and 32/96 (z) with dependency-aligned order
  (both r-halves, then both z-halves) so they pack in the PE array.
- rhn path: h_n PSUM is evacuated by a tensor_scalar(+b_hhn) and the
  r-multiply is a cheap 2x bf16 tensor_tensor (replaces the 1x
  scalar_tensor_tensor on the critical r->n segment).
"""

import os
import sys
from contextlib import ExitStack

import numpy as np

for _p in ("/opt/trn_rl_repo", "/opt/pypackages"):
    if _p not in sys.path and os.path.isdir(_p):
        sys.path.append(_p)

import concourse.bass as bass
import concourse.bacc as bacc
import concourse.tile as tile
from concourse import mybir
from concourse.bass_utils import run_bass_kernel_spmd

B, T, H = 8192, 256, 128
NCORES = 8
BS = B // NCORES  # 1024 samples per core
NT = 2            # chains
TN = BS // NT     # 512
G = 4             # steps per I/O block
F32 = mybir.dt.float32
BF16 = mybir.dt.float16
QD_NP = np.float16


def build_program(t_steps=T, bs=BS):
    assert t_steps % G == 0
    nc = bacc.Bacc("TRN2", target_bir_lowering=False, debug=False)
    gbs = G * bs

    xm = nc.dram_tensor("xm", [t_steps, 3, bs], BF16, kind="ExternalInput").ap()
    wihT = nc.dram_tensor("wihT", [H, 3 * H], BF16, kind="ExternalInput").ap()
    whhT = nc.dram_tensor("whhT", [H, 3 * H], BF16, kind="ExternalInput").ap()
    wmb = nc.dram_tensor("wmb", [H, 3 * H], BF16, kind="ExternalInput").ap()
    wquad = nc.dram_tensor("wquad", [H, 2 * H], BF16, kind="ExternalInput").ap()
    wxw = nc.dram_tensor("wxw", [H, H], BF16, kind="ExternalInput").ap()
    wsmall = nc.dram_tensor("wsmall", [H, H], BF16, kind="ExternalInput").ap()
    ident = nc.dram_tensor("ident", [H, H], BF16, kind="ExternalInput").ap()
    biases = nc.dram_tensor("biases", [H, 3], F32, kind="ExternalInput").ap()
    mone = nc.dram_tensor("mone", [1, gbs], BF16, kind="ExternalInput").ap()
    opc = nc.dram_tensor("opc", [t_steps, 2, bs], BF16, kind="ExternalOutput").ap()
    opl = nc.dram_tensor("opl", [2, bs], BF16, kind="ExternalOutput").ap()

    AF = mybir.ActivationFunctionType
    OP = mybir.AluOpType

    with tile.TileContext(nc) as tc, ExitStack() as ctx:
        const = ctx.enter_context(tc.tile_pool(name="const", bufs=1))
        work = ctx.enter_context(tc.tile_pool(name="work", bufs=2))

        # --- constants ---
        w_ih = const.tile([H, 3 * H], BF16)
        nc.sync.dma_start(w_ih[:], wihT[:])
        w_hh = const.tile([H, 3 * H], BF16)
        nc.sync.dma_start(w_hh[:], whhT[:])
        w_mb = const.tile([H, 3 * H], BF16)
        nc.sync.dma_start(w_mb[:], wmb[:])
        w_quad = const.tile([H, 2 * H], BF16)
        nc.sync.dma_start(w_quad[:], wquad[:])
        w_xw = const.tile([H, H], BF16)
        nc.sync.dma_start(w_xw[:], wxw[:])
        w_sm = const.tile([H, H], BF16)
        nc.sync.dma_start(w_sm[:], wsmall[:])
        w_id = const.tile([H, H], BF16)
        nc.sync.dma_start(w_id[:], ident[:])
        w_z = const.tile([H, H], BF16)
        nc.vector.memset(w_z[:], 0.0)
        bia = const.tile([H, 3], F32)
        nc.sync.dma_start(bia[:], biases[:])
        zrow = const.tile([2, bs], F32)
        nc.vector.memset(zrow[:], 0.0)

        # per-block staged tensors, double buffered
        m2_ab, mxc_ab, om_ab, cp_ab = [], [], [], []
        for i_ in range(2):
            mt = const.tile([H, gbs], BF16, tag=f"m2{i_}", name=f"m2f{i_}")
            nc.vector.memset(mt[:], 1.0)
            m2_ab.append(mt)
            mx = const.tile([H, gbs], BF16, tag=f"mxc{i_}", name=f"mxc{i_}")
            nc.vector.memset(mx[:], 0.0)
            mxc_ab.append(mx)
            om_ab.append(const.tile([1, gbs], BF16, tag=f"om{i_}", name=f"om{i_}"))
            cp_ab.append(const.tile([2, gbs], BF16, tag=f"cp{i_}", name=f"cp{i_}"))
        cp_l = const.tile([2, bs], BF16, tag="cpl")

        # preload I/O block 0
        for r_ in (0, 32, 64, 96):
            nc.sync.dma_start(m2_ab[0][r_:r_ + 1, :], xm[0:G, 0, :])
        nc.sync.dma_start(mxc_ab[0][1:2, :], xm[0:G, 1, :])
        nc.sync.dma_start(om_ab[0][0:1, :], xm[0:G, 2, :])

        tc.strict_bb_all_engine_barrier()

        b_hhn = bia[:, 0:1]
        b_wx = bia[:, 1:2]
        b_wc = bia[0:2, 2:3]

        h_b, ps_acc, ps_pfi, ps_phn = [], [], [], []
        for b_ in range(NT):
            hb = const.tile([H, TN], BF16, tag=f"h{b_}", name=f"h{b_}")
            nc.vector.memset(hb[:], 0.0)
            h_b.append(hb)
            ps_acc.append(ctx.enter_context(
                tc.tile_pool(name=f"psacc{b_}", bufs=1, space="PSUM")))
            ps_pfi.append(ctx.enter_context(
                tc.tile_pool(name=f"pspfi{b_}", bufs=1, space="PSUM")))
            ps_phn.append(ctx.enter_context(
                tc.tile_pool(name=f"psphn{b_}", bufs=1, space="PSUM")))

        prev_small = [None] * NT
        pending_hadd = None
        sr, sz_ = slice(0, TN), slice(TN, 2 * TN)

        # acc(0): wmb contribution issued ahead (keeps PE busy in gaps)
        acc_cur = []
        for b_ in range(NT):
            acc_cur.append(ps_acc[b_].tile([H, 2 * TN], F32, tag=f"acc{b_}",
                                           name=f"acc{b_}_0"))
        QR = (0, 64)
        QZ = (32, 96)
        for b_ in range(NT):
            r0 = QR[b_]
            nc.tensor.matmul(acc_cur[b_][:, sr], w_quad[r0:r0 + 2, 0:H],
                             m2_ab[0][r0:r0 + 2, b_ * TN:(b_ + 1) * TN],
                             start=True, stop=False, tile_position=(r0, 0))
        for b_ in range(NT):
            r1 = QZ[b_]
            nc.tensor.matmul(acc_cur[b_][:, sz_], w_quad[r1:r1 + 2, H:2 * H],
                             m2_ab[0][r1:r1 + 2, b_ * TN:(b_ + 1) * TN],
                             start=True, stop=False, tile_position=(r1, 0))

        for t in range(t_steps):
            g = t % G
            blk = (t // G) % 2
            off = g * bs
            m2 = m2_ab[blk]
            mxc = mxc_ab[blk]
            cp = cp_ab[blk]
            sob = [slice(off + b * TN, off + (b + 1) * TN) for b in range(NT)]

            # prefetch next I/O block early in this one
            if g == 1:
                t0n = (t // G + 1) * G
                if t0n < t_steps:
                    nblk = (t0n // G) % 2
                    for r_ in (0, 32, 64, 96):
                        nc.sync.dma_start(m2_ab[nblk][r_:r_ + 1, :],
                                          xm[t0n:t0n + G, 0, :])
                    nc.sync.dma_start(mxc_ab[nblk][1:2, :], xm[t0n:t0n + G, 1, :])
                    nc.sync.dma_start(om_ab[nblk][0:1, :], xm[t0n:t0n + G, 2, :])

            # drain previous completed block to HBM
            if g == 0 and t > 0:
                t0 = t - G
                pblk = (t0 // G) % 2
                nc.sync.dma_start(opc[t0:t0 + G, 0, :], cp_ab[pblk][0:1, :])
                nc.sync.dma_start(opc[t0:t0 + G, 1, :], cp_ab[pblk][1:2, :])

            # --- ACT: tanh over [clogit; pred] rows -> output stage ---
            for b in range(NT):
                if prev_small[b] is None:
                    nc.scalar.activation(cp[0:2, sob[b]], zrow[0:2, 0:TN],
                                         AF.Tanh, bias=b_wc)
                else:
                    nc.scalar.activation(cp[0:2, sob[b]], prev_small[b][0:2, :],
                                         AF.Tanh, bias=b_wc)
            # --- DVE: omc rows, with chain b1's deferred h-update add
            #     interleaved so omc(b0) is not queue-blocked behind it ---
            nc.vector.tensor_mul(mxc[0:1, sob[0]], om_ab[blk][0:1, sob[0]],
                                 cp[0:1, sob[0]])
            if pending_hadd is not None:
                nb, nn, nd = pending_hadd
                nc.vector.tensor_add(h_b[nb][:], nn[:], nd[:])
                pending_hadd = None
            nc.vector.tensor_mul(mxc[0:1, sob[1]], om_ab[blk][0:1, sob[1]],
                                 cp[0:1, sob[1]])

            # --- PE: h-dependent matmuls + featpre per chain ---
            acc = acc_cur
            h_n, featpre = [], []
            for b in range(NT):
                h_n.append(ps_phn[b].tile([H, TN], F32, tag=f"hn{b}",
                                          name=f"hn{b}_{t}"))
                featpre.append(ps_pfi[b].tile([H, TN], F32, tag=f"pfi{b}",
                                              name=f"fp{b}_{t}"))
            for b in range(NT):
                nc.tensor.matmul(h_n[b][:], w_hh[:, 2 * H:], h_b[b][:],
                                 start=True, stop=True)
                nc.tensor.matmul(acc[b][:, sr], w_hh[:, 0:H], h_b[b][:],
                                 start=False, stop=False)
                nc.tensor.matmul(acc[b][:, sz_], w_hh[:, H:2 * H], h_b[b][:],
                                 start=False, stop=False)
                nc.tensor.matmul(featpre[b][:], w_xw[:, :], mxc[0:H, sob[b]],
                                 start=True, stop=True)
            feat = []
            for b in range(NT):
                feat.append(work.tile([H, TN], BF16, tag=f"feat{b}",
                                      name=f"ft{b}_{t}"))
            nc.scalar.activation(feat[0][:], featpre[0][:], AF.Relu, bias=b_wx)
            nc.vector.tensor_scalar(feat[1][:], featpre[1][:], b_wx, 0.0,
                                    OP.add, OP.max)

            # --- feat-dependent matmuls ---
            for b in range(NT):
                nc.tensor.matmul(acc[b][:, sr], w_ih[:, 0:H], feat[b][:],
                                 start=False, stop=True)
            for b in range(NT):
                nc.tensor.matmul(acc[b][:, sz_], w_ih[:, H:2 * H], feat[b][:],
                                 start=False, stop=True)
            i_n = []
            for b in range(NT):
                i_n.append(ps_pfi[b].tile([H, TN], F32, tag=f"pfi{b}",
                                          name=f"in{b}_{t}"))
            for b in range(NT):
                nc.tensor.matmul(i_n[b][:], w_mb[:, 2 * H:], m2[0:H, sob[b]],
                                 start=True, stop=False)
            for b in range(NT):
                nc.tensor.matmul(i_n[b][:], w_ih[:, 2 * H:], feat[b][:],
                                 start=False, stop=False)

            # --- gates ---
            r_sb, z_sb = [], []
            for b in range(NT):
                r_sb.append(work.tile([H, TN], BF16, tag=f"r{b}",
                                      name=f"r{b}_{t}"))
                nc.scalar.activation(r_sb[b][:], acc[b][:, sr], AF.Sigmoid)
            for b in range(NT):
                z_sb.append(work.tile([H, TN], BF16, tag=f"z{b}",
                                      name=f"z{b}_{t}"))

            # pull next step's wmb matmuls into this step's sigma window
            if t + 1 < t_steps:
                tn = t + 1
                blkn = (tn // G) % 2
                offn = (tn % G) * bs
                m2n = m2_ab[blkn]
                acc_nx = []
                for b in range(NT):
                    acc_nx.append(ps_acc[b].tile([H, 2 * TN], F32,
                                                 tag=f"acc{b}",
                                                 name=f"acc{b}_{tn}"))
                for b in range(NT):
                    r0 = QR[b]
                    nc.tensor.matmul(acc_nx[b][:, sr], w_quad[r0:r0 + 2, 0:H],
                                     m2n[r0:r0 + 2,
                                         offn + b * TN:offn + (b + 1) * TN],
                                     start=True, stop=False,
                                     tile_position=(r0, 0))
                for b in range(NT):
                    r1 = QZ[b]
                    nc.tensor.matmul(acc_nx[b][:, sz_],
                                     w_quad[r1:r1 + 2, H:2 * H],
                                     m2n[r1:r1 + 2,
                                         offn + b * TN:offn + (b + 1) * TN],
                                     start=True, stop=False,
                                     tile_position=(r1, 0))
                acc_cur = acc_nx

            # rhn = (h_n + b_hhn) * r  (DVE), then accumulated into i_n by an
            # identity matmul on the PE; tanh_n reads the completed i_n bank.
            hn16 = []
            for b in range(NT):
                hn16.append(work.tile([H, TN], BF16, tag=f"hn16{b}", bufs=1,
                                      name=f"hc{b}_{t}"))
                nc.vector.tensor_scalar(hn16[b][:], h_n[b][:], b_hhn, None,
                                        OP.add)
            rhn = []
            for b in range(NT):
                rhn.append(work.tile([H, TN], BF16, tag=f"rhn{b}", bufs=1,
                                     name=f"rh{b}_{t}"))
                nc.vector.tensor_mul(rhn[b][:], hn16[b][:], r_sb[b][:])
            for b in range(NT):
                nc.tensor.matmul(i_n[b][:], w_id[:, :], rhn[b][:],
                                 start=False, stop=True)
            n_sb = []
            for b in range(NT):
                n_sb.append(work.tile([H, TN], BF16, tag=f"n{b}",
                                      name=f"n{b}_{t}"))
                nc.scalar.activation(z_sb[b][:], acc[b][:, sz_], AF.Sigmoid)
                nc.scalar.activation(n_sb[b][:], i_n[b][:], AF.Tanh)

            # --- h = n + z*(h-n); small = wsm@n + wsm@zd (accumulated) so
            #     the next step's tanh_c only waits for zd, not h' ---
            d_sb, small_t = [], []
            for b in range(NT):
                d_sb.append(work.tile([H, TN], BF16, tag=f"d{b}", bufs=1,
                                      name=f"d{b}_{t}"))
                small_t.append(ps_pfi[b].tile([H, TN], F32, tag=f"pfi{b}",
                                              name=f"sm{b}_{t}"))
            for b in range(NT):
                nc.vector.tensor_sub(d_sb[b][:], h_b[b][:], n_sb[b][:])
                nc.vector.tensor_mul(d_sb[b][:], d_sb[b][:], z_sb[b][:])
            nc.tensor.matmul(small_t[0][:], w_sm[:, :], n_sb[0][:],
                             start=True, stop=False)
            nc.tensor.matmul(small_t[0][:], w_sm[:, :], d_sb[0][:],
                             start=False, stop=True)
            nc.tensor.matmul(small_t[1][:], w_sm[:, :], n_sb[1][:],
                             start=True, stop=False)
            nc.tensor.matmul(small_t[1][:], w_sm[:, :], d_sb[1][:],
                             start=False, stop=True)
            for b in range(NT):
                prev_small[b] = small_t[b]
            nc.vector.tensor_add(h_b[0][:], n_sb[0][:], d_sb[0][:])
            if t + 1 < t_steps:
                pending_hadd = (1, n_sb[1], d_sb[1])
            else:
                nc.vector.tensor_add(h_b[1][:], n_sb[1][:], d_sb[1][:])
            if t + 1 < t_steps:
                for j_ in range(2):
                    nc.tensor.matmul(acc_cur[j_ % NT][:, sr],
                                     w_z[:, :], m2[0:H, sob[j_ % NT]],
                                     start=False, stop=False)

        # final: tanh of last smalls (pred_{T-1}) + remaining DMA
        for b in range(NT):
            nc.scalar.activation(cp_l[0:2, b * TN:(b + 1) * TN],
                                 prev_small[b][0:2, :], AF.Tanh, bias=b_wc)
        nc.sync.dma_start(opl[:, :], cp_l[:])
        t0 = t_steps - G
        pblk = (t0 // G) % 2
        nc.sync.dma_start(opc[t0:t0 + G, 0, :], cp_ab[pblk][0:1, :])
        nc.sync.dma_start(opc[t0:t0 + G, 1, :], cp_ab[pblk][1:2, :])

    nc.compile()
    return nc


def make_in_maps(x_seq, m_seq, Wc_w, Wc_b, Wx_w, Wx_b, W_ih, W_hh, b_ih, b_hh,
                 out_w, out_b, t_steps=T, bs=BS, ncores=NCORES):
    bf = QD_NP
    f = np.float32
    wihT = np.ascontiguousarray(W_ih[:, :H].T).astype(bf)          # [128, 384]
    whhT = np.ascontiguousarray(W_hh.T).astype(bf)                 # [128, 384]
    wquad = np.zeros((H, 2 * H), dtype=f)
    for b_, (r0, r1) in enumerate(((0, 32), (64, 96))):
        wquad[r0, 0:H] = W_ih[0:H, H]
        wquad[r0 + 1, 0:H] = b_ih[0:H] + b_hh[0:H]
        wquad[r1, H:2 * H] = W_ih[H:2 * H, H]
        wquad[r1 + 1, H:2 * H] = b_ih[H:2 * H] + b_hh[H:2 * H]
    wquad = wquad.astype(bf)
    wmb = np.zeros((H, 3 * H), dtype=f)
    wmb[0] = W_ih[:, H]
    wmb[1, 0:H] = b_ih[0:H] + b_hh[0:H]
    wmb[1, H:2 * H] = b_ih[H:2 * H] + b_hh[H:2 * H]
    wmb[1, 2 * H:] = b_ih[2 * H:]
    wmb = wmb.astype(bf)
    wxw = np.zeros((H, H), dtype=f)
    wxw[0] = Wx_w[:, 0]
    wxw[1] = Wx_w[:, 0]
    wxw = wxw.astype(bf)
    wsmall = np.zeros((H, H), dtype=f)
    wsmall[:, 0] = Wc_w[0]
    wsmall[:, 1] = out_w[0]
    wsmall = wsmall.astype(bf)
    biases = np.zeros((H, 3), dtype=f)
    biases[:, 0] = b_hh[2 * H:]
    biases[:, 1] = Wx_b
    biases[0, 2] = Wc_b[0]
    biases[1, 2] = Wc_b[0]

    xT = np.ascontiguousarray(x_seq.T, dtype=f)  # [T, B]
    mT = np.ascontiguousarray(m_seq.T, dtype=f)

    in_maps = []
    for i in range(ncores):
        sl = slice(i * bs, (i + 1) * bs)
        xmc = np.empty((t_steps, 3, bs), dtype=bf)
        xmc[:, 0, :] = mT[:t_steps, sl].astype(bf)
        xmc[:, 1, :] = (mT[:t_steps, sl] * xT[:t_steps, sl]).astype(bf)
        xmc[:, 2, :] = (1.0 - mT[:t_steps, sl]).astype(bf)
        in_maps.append({
            "xm": xmc, "wihT": wihT, "whhT": whhT, "wmb": wmb, "wxw": wxw,
            "wsmall": wsmall, "biases": biases, "wquad": wquad,
            "ident": np.eye(H, dtype=f).astype(bf),
            "mone": np.ones((1, G * bs), dtype=bf),
        })
    return in_maps


def postprocess(results, Wc_b, out_b, t_steps=T, bs=BS, ncores=NCORES):
    wc = np.float32(Wc_b[0])
    ob = np.float32(out_b[0])
    preds = np.empty((ncores * bs, t_steps), dtype=np.float32)
    comps = np.empty((ncores * bs, t_steps), dtype=np.float32)
    for i in range(ncores):
        o = np.asarray(results[i]["opc"], dtype=np.float32)  # [T, 2, bs]
        ol = np.asarray(results[i]["opl"], dtype=np.float32)  # [2, bs]
        sl = slice(i * bs, (i + 1) * bs)
        comps[sl, :] = o[:, 0, :].T
        tp = np.concatenate([o[1:, 1, :], ol[1:2, :]], axis=0)  # [T, bs]
        preds[sl, :] = np.arctanh(np.clip(tp, -0.9999999, 0.9999999)).T - wc + ob
    return preds, comps


_CACHE = {}


def kernel(x_seq, m_seq, Wc_w, Wc_b, Wx_w, Wx_b, W_ih, W_hh, b_ih, b_hh,
           out_w, out_b):
    Wc_b = np.asarray(Wc_b)
    out_b = np.asarray(out_b)
    x_seq = np.asarray(x_seq, dtype=np.float32)
    m_seq = np.asarray(m_seq, dtype=np.float32)
    if "nc" not in _CACHE:
        _CACHE["nc"] = build_program()
    nc = _CACHE["nc"]
    in_maps = make_in_maps(x_seq, m_seq, np.asarray(Wc_w), Wc_b,
                           np.asarray(Wx_w), np.asarray(Wx_b), np.asarray(W_ih),
                           np.asarray(W_hh), np.asarray(b_ih), np.asarray(b_hh),
                           np.asarray(out_w), out_b)
    res = run_bass_kernel_spmd(nc, in_maps, list(range(NCORES)))
    return postprocess(res.results, Wc_b, out_b)

